# revision 4
# baseline (speedup 1.0000x reference)
"""CirculantElman Trainium2 kernel.

Math (per batch row b, all fp32 reference):
  x_proj = x @ in_proj_w.T                      [T, 2048]
  gate   = silu(x_proj @ W_gate.T + b_gate)     [T, 2048]
  pre    = circ(c_x) @ x_proj + b  = (circ(c_x) @ in_proj_w) @ x + b
  h_t    = tanh(circ(c_h) @ h_{t-1} + pre_t)    (T sequential steps)
  out    = (h_seq * gate) @ out_proj_w.T        [T, 1024]

Distribution: pure data-parallel over batch, 1 row per NeuronCore, no
collectives.  The 2048x2048 circulant circ(c_h) is block-circulant with 16
distinct 128x128 blocks D_r[i,j] = c_h[(128 r + i - j) mod 2048]; each
recurrence step runs 16 weight-stationary matmuls (stream = 16 h-blocks in a
doubled buffer so the block rotation is a contiguous slice) plus one
identity-matmul that adds pre_t into PSUM, then tanh on ScalarE.
circ(c_x) is folded into in_proj on the host (W_pre = C_x @ in_proj_w).

Everything on-chip is feature-major ("transposed"): features on partitions,
tokens on the free axis, so matmul chains need no transposes anywhere.
"""

import os
import sys
import json

sys.path.insert(0, "/opt/trn_rl_repo")

import numpy as np
import ml_dtypes

import concourse.bass as bass
import concourse.tile as tile
from concourse import mybir
from concourse.bass import ts
from concourse.bass_utils import run_bass_kernel_spmd

F32 = mybir.dt.float32
BF16 = mybir.dt.bfloat16

# ---- config flags (iterate on precision/perf here) ----
GATE_DT = BF16      # x_proj storage + gate matmul dtype
REC_DT = F32        # recurrence weights/H dtype
PAR_DT = F32        # in_proj / pre matmul dtype (f32 or f32r)
OUT_DT = BF16       # out_proj matmul dtype (cell/w_out)

D = 2048
DIM = 1024
T = 1024
B = 8
NBLK = 16  # D / 128
KIN = 8    # DIM / 128
TCH = 512  # token chunk (one fp32 PSUM bank)
NTCH = T // TCH

LAST_EXEC_NS = None
LAST_TRACE = None


def _split_multiwaits(bir_bytes: bytes) -> bytes:
    """The walrus build here accepts only ONE sync-wait per instruction.
    Hoist extra waits onto NoOps inserted just before the owner (engine
    streams are in-order, so sequential waits == combined wait)."""
    m = json.loads(bir_bytes)
    uid = 0
    for fn in m.get("functions", []):
        for blk in fn.get("blocks", []):
            out = []
            for inst in blk.get("instructions", []):
                si = inst.get("sync_info")
                if si:
                    waits = si.get("on_wait") or []
                    if len(waits) > 1:
                        for w in waits[:-1]:
                            uid += 1
                            out.append({
                                "engine": inst["engine"],
                                "ins": [], "outs": [],
                                "name": f"WS-{uid}",
                                "opcode": "NoOp",
                                "sync_info": {"on_wait": [w], "on_update": []},
                            })
                        si["on_wait"] = [waits[-1]]
                out.append(inst)
            blk["instructions"] = out
    return json.dumps(m).encode()


def _np_dt(dt):
    return {F32: np.float32, BF16: ml_dtypes.bfloat16}[dt]


def build_nc():
    nc = bass.Bass(dynamic_dma_scratch_size=4096)

    xt = nc.dram_tensor("xt", [DIM, T], F32, kind="ExternalInput")
    h0b = nc.dram_tensor("h0b", [128, NBLK], F32, kind="ExternalInput")
    w_inT = nc.dram_tensor("w_inT", [DIM, D], GATE_DT, kind="ExternalInput")
    w_preT = nc.dram_tensor("w_preT", [DIM, D], PAR_DT, kind="ExternalInput")
    w_gateT = nc.dram_tensor("w_gateT", [D, D], GATE_DT, kind="ExternalInput")
    w_outT = nc.dram_tensor("w_outT", [D, DIM], OUT_DT, kind="ExternalInput")
    dhT = nc.dram_tensor("dhT", [128, NBLK, 128], REC_DT, kind="ExternalInput")
    ident = nc.dram_tensor("ident", [128, 128], REC_DT, kind="ExternalInput")
    bias = nc.dram_tensor("bias", [128, NBLK], F32, kind="ExternalInput")
    bgate = nc.dram_tensor("bgate", [128, NBLK], F32, kind="ExternalInput")
    outT = nc.dram_tensor("outT", [DIM, T], F32, kind="ExternalOutput")
    hfin = nc.dram_tensor("hfin", [128, NBLK], F32, kind="ExternalOutput")

    xt_r = xt[:, :].rearrange("(k p) t -> p k t", p=128)          # [128,8,T]
    w_inT_r = w_inT[:, :].rearrange("(k p) f -> p k f", p=128)    # [128,8,2048]
    w_preT_r = w_preT[:, :].rearrange("(k p) f -> p k f", p=128)
    w_gateT_r = w_gateT[:, :].rearrange("(k p) f -> p k f", p=128)  # [128,16,2048]
    w_outT_r = w_outT[:, :].rearrange("(k p) d -> p k d", p=128)    # [128,16,1024]
    outT_r = outT[:, :].rearrange("(m p) t -> p m t", p=128)        # [128,8,T]

    with tile.TileContext(nc) as tc:
        from contextlib import ExitStack
        with ExitStack() as ctx:
            # persistent (whole-kernel) pools, bottom of SBUF stack
            persist = ctx.enter_context(tc.tile_pool(name="persist", bufs=1))
            gate_sb = persist.tile([128, NBLK, T], GATE_DT, tag="gate")
            pre_sb = persist.tile([128, NBLK, T], F32, tag="pre")
            xproj_sb = persist.tile([128, NBLK, T], GATE_DT, tag="xproj")
            cell_sb = gate_sb  # cell = gate * h computed in place
            dh_sb = persist.tile([128, NBLK, 128], REC_DT, tag="dh")
            ident_sb = persist.tile([128, 128], REC_DT, tag="ident")
            bias_sb = persist.tile([128, NBLK], F32, tag="bias")
            bgate_sb = persist.tile([128, NBLK], F32, tag="bgate")
            hx = persist.tile([128, 2 * NBLK], REC_DT, tag="hx")

            nc.sync.dma_start(out=dh_sb, in_=dhT[:, :, :])
            nc.sync.dma_start(out=ident_sb, in_=ident[:, :])
            nc.sync.dma_start(out=bias_sb, in_=bias[:, :])
            nc.sync.dma_start(out=bgate_sb, in_=bgate[:, :])
            nc.sync.dma_start(out=hx[:, 0:NBLK], in_=h0b[:, :])
            nc.sync.dma_start(out=hx[:, NBLK:2 * NBLK], in_=h0b[:, :])

            # ---- Phase 1: x_projT (bf16, for gate) and preT (f32) ----
            with tc.tile_pool(name="p1x", bufs=1) as p1x, \
                 tc.tile_pool(name="p1w", bufs=2) as p1w, \
                 tc.tile_pool(name="p1ps", bufs=4, space="PSUM") as p1ps:
                xt_sb = p1x.tile([128, KIN, T], F32, tag="xt")
                nc.sync.dma_start(out=xt_sb, in_=xt_r)
                xt_g = None
                if GATE_DT != F32:
                    xt_g = p1x.tile([128, KIN, T], GATE_DT, tag="xtg")
                    for k in range(KIN):
                        nc.vector.tensor_copy(out=xt_g[:, k, :], in_=xt_sb[:, k, :])
                else:
                    xt_g = xt_sb
                for J in range(NBLK):
                    wi = p1w.tile([128, KIN, 128], GATE_DT, tag="wi")
                    wp = p1w.tile([128, KIN, 128], PAR_DT, tag="wp")
                    nc.sync.dma_start(out=wi, in_=w_inT_r[:, :, ts(J, 128)])
                    nc.sync.dma_start(out=wp, in_=w_preT_r[:, :, ts(J, 128)])
                    for tc2 in range(NTCH):
                        ps = p1ps.tile([128, TCH], F32, tag="ps")
                        for k in range(KIN):
                            nc.tensor.matmul(ps, lhsT=wi[:, k, :],
                                             rhs=xt_g[:, k, ts(tc2, TCH)],
                                             start=(k == 0), stop=(k == KIN - 1))
                        nc.vector.tensor_copy(out=xproj_sb[:, J, ts(tc2, TCH)], in_=ps)
                        ps2 = p1ps.tile([128, TCH], F32, tag="ps")
                        for k in range(KIN):
                            nc.tensor.matmul(ps2, lhsT=wp[:, k, :],
                                             rhs=xt_sb[:, k, ts(tc2, TCH)],
                                             start=(k == 0), stop=(k == KIN - 1))
                        nc.scalar.activation(out=pre_sb[:, J, ts(tc2, TCH)], in_=ps2,
                                             func=mybir.ActivationFunctionType.Identity,
                                             bias=bias_sb[:, J:J + 1], scale=1.0)

            # ---- Phase 2: gate = silu(W_gate @ x_proj + b_gate) ----
            with tc.tile_pool(name="p2w", bufs=3) as p2w, \
                 tc.tile_pool(name="p2ps", bufs=4, space="PSUM") as p2ps:
                for J in range(NBLK):
                    wg = p2w.tile([128, NBLK, 128], GATE_DT, tag="wg")
                    nc.sync.dma_start(out=wg, in_=w_gateT_r[:, :, ts(J, 128)])
                    for tc2 in range(NTCH):
                        ps = p2ps.tile([128, TCH], F32, tag="ps")
                        for k in range(NBLK):
                            nc.tensor.matmul(ps, lhsT=wg[:, k, :],
                                             rhs=xproj_sb[:, k, ts(tc2, TCH)],
                                             start=(k == 0), stop=(k == NBLK - 1))
                        nc.scalar.activation(out=gate_sb[:, J, ts(tc2, TCH)], in_=ps,
                                             func=mybir.ActivationFunctionType.Silu,
                                             bias=bgate_sb[:, J:J + 1], scale=1.0)

            # ---- Phase 3: recurrence ----
            with tc.tile_pool(name="p3ps", bufs=2, space="PSUM") as p3ps:
                for t in range(T):
                    ps = p3ps.tile([128, NBLK], F32, tag="ps")
                    for r in range(NBLK):
                        nc.tensor.matmul(ps, lhsT=dh_sb[:, r, :],
                                         rhs=hx[:, NBLK - r:2 * NBLK - r],
                                         start=(r == 0), stop=False)
                    nc.tensor.matmul(ps, lhsT=ident_sb,
                                     rhs=pre_sb[:, :, t],
                                     start=False, stop=True)
                    nc.scalar.activation(out=hx[:, NBLK:2 * NBLK], in_=ps,
                                         func=mybir.ActivationFunctionType.Tanh)
                    nc.scalar.activation(out=hx[:, 0:NBLK], in_=ps,
                                         func=mybir.ActivationFunctionType.Tanh)
                    nc.vector.tensor_mul(out=gate_sb[:, :, t],
                                         in0=gate_sb[:, :, t],
                                         in1=hx[:, NBLK:2 * NBLK])
                nc.sync.dma_start(out=hfin[:, :], in_=hx[:, NBLK:2 * NBLK])

            # ---- Phase 4: outT = W_out @ cell ----
            with tc.tile_pool(name="p4w", bufs=3) as p4w, \
                 tc.tile_pool(name="p4o", bufs=3) as p4o, \
                 tc.tile_pool(name="p4ps", bufs=4, space="PSUM") as p4ps:
                for M in range(DIM // 128):
                    wo = p4w.tile([128, NBLK, 128], OUT_DT, tag="wo")
                    nc.sync.dma_start(out=wo, in_=w_outT_r[:, :, ts(M, 128)])
                    for tc2 in range(NTCH):
                        ps = p4ps.tile([128, TCH], F32, tag="ps")
                        for k in range(NBLK):
                            nc.tensor.matmul(ps, lhsT=wo[:, k, :],
                                             rhs=cell_sb[:, k, ts(tc2, TCH)],
                                             start=(k == 0), stop=(k == NBLK - 1))
                        ot = p4o.tile([128, TCH], F32, tag="ot")
                        nc.vector.tensor_copy(out=ot, in_=ps)
                        nc.sync.dma_start(out=outT_r[:, M, ts(tc2, TCH)], in_=ot)

    # wait-splitting workaround for this walrus build
    orig = nc.to_json_bytes
    nc.to_json_bytes = lambda: _split_multiwaits(orig())
    return nc


_NC_CACHE = None


def kernel(x, h0, in_proj_w, out_proj_w, c_h, c_x, b, W_gate, b_gate):
    global _NC_CACHE, LAST_EXEC_NS, LAST_TRACE
    x = np.asarray(x, dtype=np.float32)
    h0 = np.asarray(h0, dtype=np.float32)
    in_proj_w = np.asarray(in_proj_w, dtype=np.float32)
    out_proj_w = np.asarray(out_proj_w, dtype=np.float32)
    c_h = np.asarray(c_h, dtype=np.float32)
    c_x = np.asarray(c_x, dtype=np.float32)
    b = np.asarray(b, dtype=np.float32)
    W_gate = np.asarray(W_gate, dtype=np.float32)
    b_gate = np.asarray(b_gate, dtype=np.float32)

    # ---- host prep ----
    idx = np.arange(D)
    Cx = c_x[(idx[:, None] - idx[None, :]) % D].astype(np.float64)  # [2048,2048]
    W_pre = Cx @ in_proj_w.astype(np.float64)                       # [2048,1024]
    w_preT = np.ascontiguousarray(W_pre.T).astype(_np_dt(PAR_DT))
    w_inT = np.ascontiguousarray(in_proj_w.T).astype(_np_dt(GATE_DT))
    w_gateT = np.ascontiguousarray(W_gate.T).astype(_np_dt(GATE_DT))
    w_outT = np.ascontiguousarray(out_proj_w.T).astype(_np_dt(OUT_DT))
    jj = np.arange(128)
    rr = np.arange(NBLK)
    ii = np.arange(128)
    dhT = c_h[(128 * rr[None, :, None] + ii[None, None, :] - jj[:, None, None]) % D]
    dhT = np.ascontiguousarray(dhT).astype(_np_dt(REC_DT))
    ident = np.eye(128, dtype=np.float32).astype(_np_dt(REC_DT))
    bias_blk = np.ascontiguousarray(b.reshape(NBLK, 128).T).astype(np.float32)
    bgate_blk = np.ascontiguousarray(b_gate.reshape(NBLK, 128).T).astype(np.float32)
    h0_blk = np.ascontiguousarray(h0.reshape(B, NBLK, 128).transpose(0, 2, 1)).astype(np.float32)
    xT = np.ascontiguousarray(x.transpose(0, 2, 1)).astype(np.float32)  # [B,1024,T]

    if _NC_CACHE is None:
        _NC_CACHE = build_nc()
    nc = _NC_CACHE

    shared = {
        "w_inT": w_inT, "w_preT": w_preT, "w_gateT": w_gateT, "w_outT": w_outT,
        "dhT": dhT, "ident": ident, "bias": bias_blk, "bgate": bgate_blk,
    }
    in_maps = []
    for c in range(B):
        m = dict(shared)
        m["xt"] = xT[c]
        m["h0b"] = h0_blk[c]
        in_maps.append(m)

    global _LAST_IN_MAPS
    _LAST_IN_MAPS = in_maps
    trace = bool(int(os.environ.get("KERNEL_TRACE", "0")))
    res = run_bass_kernel_spmd(nc, in_maps, core_ids=list(range(B)), trace=trace)
    LAST_EXEC_NS = res.exec_time_ns
    LAST_TRACE = res.instructions_and_trace[1] if res.instructions_and_trace else None

    out = np.empty((B, T, DIM), dtype=np.float32)
    h_final = np.empty((B, D), dtype=np.float32)
    for c in range(B):
        out[c] = res.results[c]["outT"].T
        h_final[c] = res.results[c]["hfin"].T.reshape(-1)
    return out, h_final
